# revision 21
# baseline (speedup 1.0000x reference)
"""Trainium2 Bass kernel for nn_DiffTopkNet (soft bitonic top-k), v2.

Strategy
--------
Data parallel over 8 cores (32 batch rows each).  SBUF partitions =
4 column-chunks x 32 batch rows.  Per core:

1. Forward pass over x [32, 512] through the 45 bitonic compare-swap
   layers with the centered-alpha factorization:
       d  = v - u;  ds = d*s          (s = +-1 per pair block)
       A  = arctan(10*ds) = AB_lo - AB_hi + sgn(ds)*pi/4
       beta = A/pi - 1/2               (the backward coefficient)
       dst_u = u - beta*d;  dst_v = v + beta*d
   The two-branch arctan keeps the ACT LUT inputs inside [-1, 1]:
       AB = Arctan(10*[clamp(ds,.1) | clamp(0.01/ds,.1)])
   The reciprocal is an all-Pool fast-inverse on |d| (magic-constant
   seed + one Newton step, int32 bitcast TSP) -- ISA-legal (no STT, no
   divide) and tie-safe (|d|=0 saturates to the clamped branch).  Pool
   runs 13 ops/layer; the sign handling (ds, sgn, s*|d|, u+d/2) and
   the deferred beta16 = sgn(ds)*Cq - 1/2 write run on DVE
   concurrently.  Measured period: 1128 ns/layer (CoreSim).

2. Backward pass: evolve Y [16, 512] f16 from the selector rows
   through the layers in reverse (U' = U - beta*dY, V' = V + beta*dY),
   4 tensor_tensor ops per layer k-split 9/7 across DVE (f16 2x mode)
   and Pool.  First 6 layers touch only the selector support
   (in-place); cross-chunk layers t=37,36 use the zero-support form
   U' = -beta*V, V' = (1+beta)*V with coefficients pre-staged on the
   right partitions; t=28 moves the v-chunk via SBUF-SBUF DMA.  The
   last layer writes f32; the output DMA is split per (chunk, k-half)
   across the three DMA-capable queues.
"""

import numpy as np

BATCH, SIZE, K, NCORES = 256, 512, 16, 8
BC = BATCH // NCORES          # 32 batch rows per core
NL = 45                       # bitonic layers for n=512
PI = float(np.pi)


def _layers():
    out = []
    k = 2
    while k <= SIZE:
        j = k // 2
        while j >= 1:
            out.append((k, j))
            j //= 2
        k *= 2
    return out


LAYERS = _layers()
SPECIALS = [t for t, (k, j) in enumerate(LAYERS) if j >= 128]  # [28, 36, 37]
SGN_COLS = NL * 64


def _sgn_table():
    """[128, SGN_COLS] f16: s = +-1 per (chunk-partition, pair index)."""
    sgn = np.ones((128, SGN_COLS), np.float16)
    for t, (k, j) in enumerate(LAYERS):
        if j > 64:
            continue
        m = np.arange(64)
        for c in range(4):
            base = c * 128 + (m // j) * 2 * j + (m % j)
            s = np.where((base & k) == 0, 1.0, -1.0)
            sgn[c * 32:(c + 1) * 32, 64 * t:64 * t + 64] = s[None, :].astype(np.float16)
    return sgn


def build_nc():
    import concourse.bacc as bacc
    import concourse.mybir as mybir
    from concourse import tile

    f32 = mybir.dt.float32
    f16 = mybir.dt.float16
    i32 = mybir.dt.int32
    AT = mybir.ActivationFunctionType
    OP = mybir.AluOpType

    nc = bacc.Bacc("TRN2", target_bir_lowering=False, debug=False, num_devices=1)
    x_d = nc.dram_tensor("x", [BC, SIZE], f32, kind="ExternalInput")
    sg_d = nc.dram_tensor("sgn", [128, SGN_COLS], f16, kind="ExternalInput")
    y_d = nc.dram_tensor("y", [BC, K, SIZE], f32, kind="ExternalOutput")

    with tile.TileContext(nc) as tc:
        with tc.tile_pool(name="persist", bufs=1) as pp, \
             tc.tile_pool(name="scratch", bufs=3) as sp:
            xA = pp.tile([128, 128], f32)
            xB = pp.tile([128, 128], f32)
            yA = pp.tile([128, K * 128], f16)
            yB = pp.tile([128, K * 128], f16)
            yF = pp.tile([128, K * 128], f32)
            sgn_t = pp.tile([128, SGN_COLS], f16)
            b16 = pp.tile([128, NL * 64], f16)
            b16s = pp.tile([128, len(SPECIALS) * 128], f16)
            bneg16 = pp.tile([128, len(SPECIALS) * 128], f16)
            b1p16 = pp.tile([128, len(SPECIALS) * 128], f16)

            nc.sync.dma_start(xA[:], x_d[:].rearrange("b (c i) -> c b i", c=4))
            H0 = 4 * 64
            H = SGN_COLS // 2
            nc.sync.dma_start(sgn_t[:, :H0], sg_d[:, :H0])
            nc.sync.dma_start(sgn_t[:, H0:H], sg_d[:, H0:H])
            nc.sync.dma_start(sgn_t[:, H:], sg_d[:, H:])

            xs = [xA, xB]

            # per-layer state threaded from pre(t) to tail(t):
            # (d, u2, v2, b4, cs, AB, dst_views)
            fwd_state = {}

            MAGIC = 0x7EF311C3

            def fwd_pre(t, j, src, dst):
                sv = src[:].rearrange("p (nb two j) -> p nb two j", two=2, j=j)
                u, v = sv[:, :, 0, :], sv[:, :, 1, :]
                d = sp.tile([128, 64], f32, name="d")
                ds = sp.tile([128, 64], f32, name="ds")
                ads = sp.tile([128, 64], f32, name="ads")
                ri = sp.tile([128, 64], f32, name="ri")
                tq = sp.tile([128, 64], f32, name="tq")
                uq = sp.tile([128, 64], f32, name="uq")
                w = sp.tile([128, 64], f32, name="w")
                cs = sp.tile([128, 128], f32, name="cs")
                AB = sp.tile([128, 128], f32, name="AB")
                fb = sp.tile([128, 64], f32, name="fb")
                sad = sp.tile([128, 64], f32, name="sad")
                dh = sp.tile([128, 64], f32, name="dh")
                sav = sp.tile([128, 64], f32, name="sav")
                d_v = d[:].rearrange("p (nb j) -> p nb j", j=j)
                dh_v = dh[:].rearrange("p (nb j) -> p nb j", j=j)
                sav_v = sav[:].rearrange("p (nb j) -> p nb j", j=j)
                sg = sgn_t[:, 64 * t:64 * t + 64]
                P = nc.gpsimd
                # critical pre-chain: |d|-domain fast-inverse (magic seed +
                # one Newton step), sign handling is entirely off-path.
                P.tensor_tensor(d_v, v, u, op=OP.subtract)
                P.tensor_scalar(ads[:], d[:], 0.0, None, op0=OP.abs_max)
                P.tensor_scalar(ri[:].bitcast(i32), ads[:].bitcast(i32), MAGIC, -1,
                                op0=OP.subtract, op1=OP.mult)
                P.tensor_tensor(tq[:], ads[:], ri[:], op=OP.mult)
                P.tensor_scalar(uq[:], tq[:], 2.0, None, op0=OP.subtract)
                P.tensor_tensor(w[:], ri[:], uq[:], op=OP.mult)
                P.tensor_scalar(cs[:, 64:], w[:], 0.01, -0.1, op0=OP.mult, op1=OP.max)
                # off-path on DVE (runs concurrently with the Pool chain);
                # cs_lo needs only ads, so DVE computes it while Pool runs
                # the Newton chain.  fb = sgn(ds) = 4*b4 - 1 is Cq-independent
                # so the late beta write (deferred) is only 2 DVE ops.
                V = nc.vector
                V.tensor_scalar(cs[:, :64], ads[:], 0.1, None, op0=OP.min)
                nc.scalar.activation(AB[:], cs[:], AT.Arctan, scale=10.0)
                V.tensor_tensor(ds[:], d[:], sg, op=OP.mult)
                V.tensor_scalar(fb[:], ds[:], 0.0, 2.0, op0=OP.is_ge, op1=OP.mult)
                V.tensor_scalar(fb[:], fb[:], 1.0, None, op0=OP.subtract)
                V.tensor_tensor(sad[:], ads[:], sg, op=OP.mult)
                V.tensor_scalar(dh[:], d[:], 0.5, None, op0=OP.mult)
                V.tensor_tensor(sav_v, u, dh_v, op=OP.add)
                if beta_state:
                    _flush_beta()
                dv_ = dst[:].rearrange("p (nb two j) -> p nb two j", two=2, j=j)
                fwd_state[t] = (sad, sav, fb, AB, dv_[:, :, 0, :], dv_[:, :, 1, :], j)

            beta_state = []

            def _flush_beta():
                t0, fb0, Cq0 = beta_state.pop()
                m1 = sp.tile([128, 64], f32, name="m1")
                nc.vector.tensor_tensor(m1[:], fb0[:], Cq0[:], op=OP.mult)
                nc.vector.tensor_scalar(b16[:, 64 * t0:64 * t0 + 64], m1[:],
                                        0.5, None, op0=OP.subtract)

            def fwd_tail(t):
                sad, sav, fb, AB, du_v, dv_v, j = fwd_state.pop(t)
                t1 = sp.tile([128, 64], f32, name="t1")
                Cq = sp.tile([128, 64], f32, name="Cq")
                T1 = sp.tile([128, 64], f32, name="T1")
                T1_v = T1[:].rearrange("p (nb j) -> p nb j", j=j)
                sav_v = sav[:].rearrange("p (nb j) -> p nb j", j=j)
                P = nc.gpsimd
                # A = sgn(ds)*(G_lo - G_hi + pi/4);  AB_hi holds -G_hi
                P.tensor_tensor(t1[:], AB[:, :64], AB[:, 64:], op=OP.add)
                P.tensor_scalar(Cq[:], t1[:], 1.0 / PI, 0.25, op0=OP.mult, op1=OP.add)
                P.tensor_tensor(T1[:], Cq[:], sad[:], op=OP.mult)
                P.tensor_tensor(du_v, sav_v, T1_v, op=OP.subtract)
                P.tensor_tensor(dv_v, sav_v, T1_v, op=OP.add)
                # beta16 = sgn(ds)*Cq - 1/2, deferred to the next layer's DVE slot
                beta_state.append((t, fb, Cq))

            def fwd_special(si, t, j, src, dst):
                if beta_state:
                    _flush_beta()
                # Cross-chunk layer: v-chunk shuffled onto the u-partitions,
                # the alpha-centered chain at [*,128], results shuffled back.
                # Signs are uniform per group: t=28 g0:+1 g1:-1; t=36/37 all +1.
                k = LAYERS[t][0]
                groups = ([(slice(0, 32), slice(32, 64)), (slice(64, 96), slice(96, 128))]
                          if j == 128 else [(slice(0, 64), slice(64, 128))])
                col = 128 * si
                vt = sp.tile([128, 128], f32, name="vt")
                d = sp.tile([128, 128], f32, name="d_s")
                cs = sp.tile([128, 256], f32, name="cs_s")
                AB = sp.tile([128, 256], f32, name="AB_s")
                ads = sp.tile([128, 128], f32, name="ads_s")
                ri = sp.tile([128, 128], f32, name="ri_s")
                tq = sp.tile([128, 128], f32, name="tq_s")
                uq = sp.tile([128, 128], f32, name="uq_s")
                w = sp.tile([128, 128], f32, name="w_s")
                b4 = sp.tile([128, 128], f32, name="b4_s")
                sad = sp.tile([128, 128], f32, name="sad_s")
                dh = sp.tile([128, 128], f32, name="dh_s")
                sav = sp.tile([128, 128], f32, name="sav_s")
                t1 = sp.tile([128, 128], f32, name="t1_s")
                Cq = sp.tile([128, 128], f32, name="Cq_s")
                T1 = sp.tile([128, 128], f32, name="T1_s")
                bb = sp.tile([128, 128], f32, name="bb_s")
                z1 = sp.tile([128, 128], f32, name="z1_s")
                ntv = sp.tile([128, 128], f32, name="ntv")
                b1tmp = sp.tile([128, 128], f16, name="b1tmp")
                for pu, pv in groups:
                    nc.vector.stream_shuffle(vt[pu, :], src[pv, :], mask=list(range(32)))
                # phase A: both groups' pre-chains + ACTs (Pool of group 1
                # overlaps group 0's ACT round trip); sign ops on DVE.
                for gi, (pu, pv) in enumerate(groups):
                    base0 = (pu.start // 32) * 128
                    neg = ((base0 & k) != 0)
                    s = -1.0 if neg else 1.0
                    P = nc.gpsimd
                    V = nc.vector
                    P.tensor_tensor(d[pu, :], vt[pu, :], src[pu, :], op=OP.subtract)
                    P.tensor_scalar(ads[pu, :], d[pu, :], 0.0, None, op0=OP.abs_max)
                    P.tensor_scalar(ri[pu, :].bitcast(i32), ads[pu, :].bitcast(i32),
                                    MAGIC, -1, op0=OP.subtract, op1=OP.mult)
                    P.tensor_scalar(cs[pu, 128:], ri[pu, :], -0.01, -0.1,
                                    op0=OP.mult, op1=OP.max)
                    P.tensor_scalar(cs[pu, :128], ads[pu, :], 0.1, None, op0=OP.min)
                    nc.scalar.activation(AB[pu, :], cs[pu, :], AT.Arctan, scale=10.0)
                    # ds = s*d; sad = sgn(ds)*d = s*|d|
                    V.tensor_scalar(b4[pu, :], d[pu, :], 0.0, 0.5 * s,
                                    op0=OP.is_ge, op1=OP.mult)
                    if neg:
                        V.tensor_scalar(b4[pu, :], b4[pu, :], 0.5, None, op0=OP.add)
                    V.tensor_scalar(sad[pu, :], ads[pu, :], s, None, op0=OP.mult)
                    V.tensor_scalar(dh[pu, :], d[pu, :], 0.5, None, op0=OP.mult)
                    V.tensor_tensor(sav[pu, :], src[pu, :], dh[pu, :], op=OP.add)
                # phase B: tails
                for gi, (pu, pv) in enumerate(groups):
                    P = nc.gpsimd
                    P.tensor_tensor(t1[pu, :], AB[pu, :128], AB[pu, 128:], op=OP.add)
                    P.tensor_scalar(Cq[pu, :], t1[pu, :], 1.0 / PI, 0.25,
                                    op0=OP.mult, op1=OP.add)
                    P.tensor_tensor(T1[pu, :], Cq[pu, :], sad[pu, :], op=OP.mult)
                    P.tensor_tensor(dst[pu, :], sav[pu, :], T1[pu, :], op=OP.subtract)
                    P.tensor_tensor(ntv[pu, :], sav[pu, :], T1[pu, :], op=OP.add)
                    # beta16 = 4*b4*Cq - Cq - 1/2 (b4 here = 0.5*[ds>=0])
                    nc.vector.tensor_tensor(bb[pu, :], b4[pu, :], Cq[pu, :], op=OP.mult)
                    nc.vector.tensor_scalar(z1[pu, :], bb[pu, :], 4.0, -0.5,
                                            op0=OP.mult, op1=OP.add)
                    nc.vector.tensor_tensor(b16s[pu, col:col + 128], z1[pu, :],
                                            Cq[pu, :], op=OP.subtract)
                    if t != 28:
                        nc.vector.tensor_scalar(bneg16[pu, col:col + 128],
                                                b16s[pu, col:col + 128], -1.0, None,
                                                op0=OP.mult)
                        nc.vector.tensor_scalar(b1tmp[pu, :],
                                                b16s[pu, col:col + 128], 1.0, None,
                                                op0=OP.add)
                for pu, pv in groups:
                    nc.vector.stream_shuffle(dst[pv, :], ntv[pu, :], mask=list(range(32)))
                    if t != 28:
                        nc.vector.stream_shuffle(b1p16[pv, col:col + 128],
                                                 b1tmp[pu, :], mask=list(range(32)))

            prev_t = None
            for t, (k, j) in enumerate(LAYERS):
                src, dst = xs[t % 2], xs[(t + 1) % 2]
                if j <= 64:
                    if prev_t is not None:
                        fwd_tail(prev_t)
                    fwd_pre(t, j, src, dst)
                    prev_t = t
                else:
                    if prev_t is not None:
                        fwd_tail(prev_t)
                        prev_t = None
                    fwd_special(SPECIALS.index(t), t, j, src, dst)
            if prev_t is not None:
                fwd_tail(prev_t)
            if beta_state:
                _flush_beta()

            # ---- backward over Y [16 x 512] per batch row ----
            nc.vector.memset(yA[:], 0.0)
            nc.gpsimd.memset(yA[:][96:128, 127:K * 128:127], 1.0)
            ys = [yA, yB]

            KSPLIT = ((nc.vector, 0, 9), (nc.gpsimd, 9, K))

            def bwd_normal(t, j, src, dst):
                ksplit = ((nc.vector, 0, 8), (nc.gpsimd, 8, K)) if j == 1 else KSPLIT
                nb = 64 // j
                sv = src[:].rearrange("p (k nb two j) -> p k nb two j", k=K, two=2, j=j)
                dv = dst[:].rearrange("p (k nb two j) -> p k nb two j", k=K, two=2, j=j)
                dY = sp.tile([128, K * 64], f16, name="dY")
                nwb = sp.tile([128, K * 64], f16, name="nwb")
                dY_v = dY[:].rearrange("p (k nb j) -> p k nb j", k=K, j=j)
                nwb_v = nwb[:].rearrange("p (k nb j) -> p k nb j", k=K, j=j)
                for eng, klo, khi in ksplit:
                    kc = khi - klo
                    YU = sv[:, klo:khi, :, 0, :]
                    YV = sv[:, klo:khi, :, 1, :]
                    dYs = dY_v[:, klo:khi]
                    nwbs = nwb_v[:, klo:khi]
                    g_bc = (b16[:, 64 * t:64 * t + 64]
                            .rearrange("p (o nb j) -> p o nb j", o=1, j=j)
                            .broadcast_to([128, kc, nb, j]))
                    eng.tensor_tensor(dYs, YV, YU, op=OP.subtract)
                    eng.tensor_tensor(nwbs, g_bc, dYs, op=OP.mult)
                    eng.tensor_tensor(dv[:, klo:khi, :, 0, :], YU, nwbs, op=OP.subtract)
                    eng.tensor_tensor(dv[:, klo:khi, :, 1, :], YV, nwbs, op=OP.add)

            def bwd_sparse(t, j, nb0, nbc, tile_):
                # Support-limited early backward layers: only chunk-3
                # partitions and nbc blocks are nonzero; update in place.
                w = K * nbc * j
                sv = tile_[96:128, :].rearrange("p (k nb two j) -> p k nb two j",
                                                k=K, two=2, j=j)
                YU = sv[:, :, nb0:nb0 + nbc, 0, :]
                YV = sv[:, :, nb0:nb0 + nbc, 1, :]
                dY = sp.tile([128, K * 64], f16, name="dY")
                nwb = sp.tile([128, K * 64], f16, name="nwb")
                dY_v = dY[96:128, :w].rearrange("p (k nb j) -> p k nb j", k=K, j=j)
                nwb_v = nwb[96:128, :w].rearrange("p (k nb j) -> p k nb j", k=K, j=j)
                for eng, klo, khi in KSPLIT:
                    kc = khi - klo
                    g_bc = (b16[96:128, 64 * t + nb0 * j:64 * t + (nb0 + nbc) * j]
                            .rearrange("p (o nb j) -> p o nb j", o=1, j=j)
                            .broadcast_to([32, kc, nbc, j]))
                    eng.tensor_tensor(dY_v[:, klo:khi], YV[:, klo:khi],
                                      YU[:, klo:khi], op=OP.subtract)
                    eng.tensor_tensor(nwb_v[:, klo:khi], g_bc,
                                      dY_v[:, klo:khi], op=OP.mult)
                    eng.tensor_tensor(YU[:, klo:khi], YU[:, klo:khi],
                                      nwb_v[:, klo:khi], op=OP.subtract)
                    eng.tensor_tensor(YV[:, klo:khi], YV[:, klo:khi],
                                      nwb_v[:, klo:khi], op=OP.add)

            def bwd_special(si, t, j, src, dst):
                groups = ([(slice(0, 32), slice(32, 64)), (slice(64, 96), slice(96, 128))]
                          if j == 128 else [(slice(0, 64), slice(64, 128))])
                yvt = sp.tile([128, K * 128], f16, name="yvt")
                dY = sp.tile([128, K * 128], f16, name="dY_s")
                nwb = sp.tile([128, K * 128], f16, name="nwb_s")
                nyt = sp.tile([128, K * 128], f16, name="nyt")
                for gi, (pu, pv) in enumerate(groups):
                    (nc.sync if gi == 0 else nc.scalar).dma_start(
                        yvt[pu, :9 * 128], src[pv, :9 * 128])
                    (nc.scalar if gi == 0 else nc.sync).dma_start(
                        yvt[pu, 9 * 128:], src[pv, 9 * 128:])
                for pu, pv in groups:
                    L = pu.stop - pu.start
                    sv_u = src[pu, :].rearrange("p (k i) -> p k i", k=K)
                    vv = yvt[pu, :].rearrange("p (k i) -> p k i", k=K)
                    dY_v = dY[pu, :].rearrange("p (k i) -> p k i", k=K)
                    nwb_v = nwb[pu, :].rearrange("p (k i) -> p k i", k=K)
                    du_v = dst[pu, :].rearrange("p (k i) -> p k i", k=K)
                    for eng, klo, khi in KSPLIT:
                        kc = khi - klo
                        g_bc = (b16s[pu, 128 * si:128 * si + 128]
                                .rearrange("p (o i) -> p o i", o=1)
                                .broadcast_to([L, kc, 128]))
                        eng.tensor_tensor(dY_v[:, klo:khi], vv[:, klo:khi],
                                          sv_u[:, klo:khi], op=OP.subtract)
                        eng.tensor_tensor(nwb_v[:, klo:khi], g_bc,
                                          dY_v[:, klo:khi], op=OP.mult)
                        eng.tensor_tensor(du_v[:, klo:khi], sv_u[:, klo:khi],
                                          nwb_v[:, klo:khi], op=OP.subtract)
                for gi, (pu, pv) in enumerate(groups):
                    (nc.sync if gi == 0 else nc.scalar).dma_start(
                        nyt[pv, :9 * 128], nwb[pu, :9 * 128])
                    (nc.scalar if gi == 0 else nc.sync).dma_start(
                        nyt[pv, 9 * 128:], nwb[pu, 9 * 128:])
                for pu, pv in groups:
                    sv_v = src[pv, :].rearrange("p (k i) -> p k i", k=K)
                    dv_v = dst[pv, :].rearrange("p (k i) -> p k i", k=K)
                    ny_v = nyt[pv, :].rearrange("p (k i) -> p k i", k=K)
                    for eng, klo, khi in KSPLIT:
                        eng.tensor_tensor(dv_v[:, klo:khi], sv_v[:, klo:khi],
                                          ny_v[:, klo:khi], op=OP.add)

            def bwd_special_zero(si, pu, pv, src, dst):
                # Base chunks (pu) all-zero:  newU = -beta*V (on pu, needs V
                # shuffled over),  newV = (1+beta)*V (in place on pv, with the
                # coefficient pre-staged on pv during the forward pass).
                L = pu.stop - pu.start
                yvt = sp.tile([128, K * 128], f16, name="yvt")
                nc.sync.dma_start(yvt[pu, :K * 64], src[pv, :K * 64])
                nc.scalar.dma_start(yvt[pu, K * 64:], src[pv, K * 64:])
                vv = yvt[pu, :].rearrange("p (k i) -> p k i", k=K)
                du_v = dst[pu, :].rearrange("p (k i) -> p k i", k=K)
                sv_v = src[pv, :].rearrange("p (k i) -> p k i", k=K)
                dv_v = dst[pv, :].rearrange("p (k i) -> p k i", k=K)
                for eng, klo, khi in KSPLIT:
                    kc = khi - klo
                    gneg = (bneg16[pu, 128 * si:128 * si + 128]
                            .rearrange("p (o i) -> p o i", o=1)
                            .broadcast_to([L, kc, 128]))
                    g1p = (b1p16[pv, 128 * si:128 * si + 128]
                           .rearrange("p (o i) -> p o i", o=1)
                           .broadcast_to([L, kc, 128]))
                    eng.tensor_tensor(dv_v[:, klo:khi], sv_v[:, klo:khi], g1p,
                                      op=OP.mult)
                    eng.tensor_tensor(du_v[:, klo:khi], vv[:, klo:khi], gneg,
                                      op=OP.mult)

            # t -> (first block, n blocks) of the nonzero support
            SPARSE = {44: (56, 8), 43: (28, 4), 42: (14, 2),
                      41: (7, 1), 40: (3, 1), 39: (1, 1)}
            for t in range(NL - 1, NL - 1 - len(SPARSE), -1):
                k, j = LAYERS[t]
                nb0, nbc = SPARSE[t]
                bwd_sparse(t, j, nb0, nbc, yA)
            s2 = 0
            for t in range(NL - 1 - len(SPARSE), -1, -1):
                k, j = LAYERS[t]
                src, dst = ys[s2 % 2], ys[(s2 + 1) % 2]
                if t == 0:
                    dst = yF
                s2 += 1
                if j <= 64:
                    bwd_normal(t, j, src, dst)
                elif t == 37:
                    # k=512, j=128: chunks 0,1 zero -> (c0,c1) group is a
                    # no-op (dst rows 0:64 already zero); (c2,c3) is zero-base
                    bwd_special_zero(SPECIALS.index(t), slice(64, 96),
                                     slice(96, 128), src, dst)
                elif t == 36:
                    # k=512, j=256: base chunks 0,1 zero
                    bwd_special_zero(SPECIALS.index(t), slice(0, 64),
                                     slice(64, 128), src, dst)
                else:
                    bwd_special(SPECIALS.index(t), t, j, src, dst)

            # Output DMA split per (chunk, k-half): DRAM-side AP leads with
            # b=32 and keeps 512B contiguous runs; spreading across the three
            # DMA-capable engines overlaps the transfers.
            dma_engs = [nc.sync, nc.scalar, nc.gpsimd]
            di = 0
            for c in range(4):
                for kh in range(2):
                    dst = y_d[:, kh * 8:(kh + 1) * 8, c * 128:(c + 1) * 128]
                    src = (yF[32 * c:32 * (c + 1), :]
                           .rearrange("b (k i) -> b k i", k=K)[:, kh * 8:(kh + 1) * 8, :])
                    dma_engs[di % 3].dma_start(dst, src)
                    di += 1

    nc.compile()
    return nc


_NC_CACHE = {}


def _get_nc():
    if "nc" not in _NC_CACHE:
        _NC_CACHE["nc"] = build_nc()
    return _NC_CACHE["nc"]


def _run_hw(vectors: np.ndarray) -> np.ndarray:
    from concourse.bass_utils import run_bass_kernel_spmd

    nc = _get_nc()
    sgn = _sgn_table()
    in_maps = [{"x": np.ascontiguousarray(vectors[c * BC:(c + 1) * BC]), "sgn": sgn}
               for c in range(NCORES)]
    res = run_bass_kernel_spmd(nc, in_maps, core_ids=list(range(NCORES)))
    out = np.empty((BATCH, K, SIZE), np.float32)
    for c in range(NCORES):
        out[c * BC:(c + 1) * BC] = res.results[c]["y"].reshape(BC, K, SIZE)
    return out


def _hw_worker(infile: str, outfile: str) -> None:
    vec = np.load(infile)
    np.save(outfile, _run_hw(vec))


def _run_sim(vectors: np.ndarray) -> np.ndarray:
    """Bit-exact local fallback (CoreSim) when the device path is unavailable."""
    from concourse.bass_interp import CoreSim

    nc = _get_nc()
    sgn = _sgn_table()
    out = np.empty((BATCH, K, SIZE), np.float32)
    for c in range(NCORES):
        sim = CoreSim(nc, require_finite=False, require_nnan=False,
                      publish_trace=False)
        sim.tensor("x")[:] = vectors[c * BC:(c + 1) * BC]
        sim.tensor("sgn")[:] = sgn
        sim.simulate()
        out[c * BC:(c + 1) * BC] = np.array(sim.tensor("y")).reshape(BC, K, SIZE)
    return out


def kernel(vectors: np.ndarray) -> np.ndarray:
    import os
    import subprocess
    import sys
    import tempfile

    vectors = np.asarray(vectors, np.float32)
    assert vectors.shape == (BATCH, SIZE)

    # Hardware attempt in a watchdog subprocess: a wedged device tunnel can
    # hang an in-process PJRT execute forever; a subprocess we can time out.
    here = os.path.dirname(os.path.abspath(__file__))
    with tempfile.TemporaryDirectory() as td:
        inf = os.path.join(td, "in.npy")
        outf = os.path.join(td, "out.npy")
        np.save(inf, vectors)
        code = (
            "import sys; sys.path.insert(0, %r); "
            "import kernel; kernel._hw_worker(%r, %r)" % (here, inf, outf)
        )
        try:
            proc = subprocess.run(
                [sys.executable, "-c", code],
                timeout=int(os.environ.get("KERNEL_HW_TIMEOUT", "900")),
                capture_output=True,
            )
            if proc.returncode == 0 and os.path.exists(outf):
                return np.load(outf)
            sys.stderr.write(
                "kernel: hw subprocess failed (rc=%s); falling back to CoreSim\n%s\n"
                % (proc.returncode, proc.stderr.decode(errors="replace")[-2000:])
            )
        except subprocess.TimeoutExpired:
            sys.stderr.write("kernel: hw subprocess timed out; falling back to CoreSim\n")
    return _run_sim(vectors)
